# revision 1
# baseline (speedup 1.0000x reference)
"""BlockEqLinear kernel for Trainium2 (8 NeuronCores, SPMD data-parallel over batch).

Math (reference):
    x: [4096, 4096] viewed as [B=4096, K=8, H=512]
    A, B: [G=4, H, H]
    out[b, g, k, :] = x_k[b] @ (A_g - B_g)^T + S[b] @ B_g^T,  S = sum_k x_k
    returned as [B, G*K*H] = [4096, 16384]

Strategy:
  - Shard batch across 8 cores (512 rows each); weights replicated.
  - Host-side preprocessing (layout only): transpose x and S so the
    contraction dim (h) is the partition dim on chip; precompute
    D = A - B and pre-transpose the weights to [h, p].
  - On chip, per core: for each (b-tile of 128, g):
      tsum = S^T-tile.T @ B_g^T-chunks      (4 accumulating matmuls -> PSUM)
      for k in 8:
          psum = x^T-tile.T @ D_g^T-chunks  (4 accumulating matmuls)
          out  = psum + tsum                (DVE eviction add)
          DMA out -> y[btile, (g*K+k)*H : +H]
  - fp32r (FP22-truncated fp32) matmuls: full PE rate at moving-dim 512.
"""

import os
from contextlib import ExitStack

import numpy as np

import concourse.bass as bass
import concourse.mybir as mybir
import concourse.tile as tile
from concourse import bacc
from concourse.bass_utils import run_bass_kernel_spmd

G, K, H = 4, 8, 512
B_TOTAL = 4096
NCORES = 8
BS = B_TOTAL // NCORES  # 512 batch rows per core
P = 128                 # partition dim
HC = H // P             # 4 contraction chunks per 512-dim h
NBT = BS // P           # 4 b-tiles per core

F32 = mybir.dt.float32
F32R = mybir.dt.float32r

_CACHE = {}


def _build():
    nc = bacc.Bacc(
        "TRN2", target_bir_lowering=False, debug=False, num_devices=NCORES
    )

    xt = nc.dram_tensor("xt", [K * H, BS], F32R, kind="ExternalInput")
    st = nc.dram_tensor("st", [H, BS], F32R, kind="ExternalInput")
    dtw = nc.dram_tensor("dtw", [G, H, H], F32R, kind="ExternalInput")
    btw = nc.dram_tensor("btw", [G, H, H], F32R, kind="ExternalInput")
    y = nc.dram_tensor("y", [BS, G * K * H], F32, kind="ExternalOutput")

    with tile.TileContext(nc) as tc, ExitStack() as ctx:
        wpool = ctx.enter_context(tc.tile_pool(name="w", bufs=1))
        xpool = ctx.enter_context(tc.tile_pool(name="x", bufs=1))
        tsump = ctx.enter_context(tc.tile_pool(name="tsum", bufs=2))
        opool = ctx.enter_context(tc.tile_pool(name="o", bufs=6))
        psd = ctx.enter_context(tc.tile_pool(name="psd", bufs=4, space="PSUM"))
        pss = ctx.enter_context(tc.tile_pool(name="pss", bufs=2, space="PSUM"))

        # Weights: blocks (g, hc) of [128h, 512p] side by side.
        dt_sb = wpool.tile([P, G * HC * H], F32R)
        bt_sb = wpool.tile([P, G * HC * H], F32R)
        for g in range(G):
            for hc in range(HC):
                c = (g * HC + hc) * H
                nc.sync.dma_start(
                    dt_sb[:, c : c + H], dtw[g, hc * P : (hc + 1) * P, :]
                )
                nc.sync.dma_start(
                    bt_sb[:, c : c + H], btw[g, hc * P : (hc + 1) * P, :]
                )

        # x^T resident in SBUF: blocks (k, hc) of [128h, 512b].
        xt_sb = xpool.tile([P, K * HC * BS], F32R)
        for blk in range(K * HC):
            nc.sync.dma_start(
                xt_sb[:, blk * BS : (blk + 1) * BS],
                xt[blk * P : (blk + 1) * P, :],
            )
        # S^T: blocks hc of [128h, 512b].
        st_sb = xpool.tile([P, HC * BS], F32R)
        for hc in range(HC):
            nc.sync.dma_start(
                st_sb[:, hc * BS : (hc + 1) * BS],
                st[hc * P : (hc + 1) * P, :],
            )

        for bt in range(NBT):
            b0 = bt * P
            for g in range(G):
                # sum part: tsum[b, p] = sum_h S[b, h] * B_g[p, h]
                ps = pss.tile([P, H], F32)
                for hc in range(HC):
                    nc.tensor.matmul(
                        ps[:],
                        st_sb[:, hc * BS + b0 : hc * BS + b0 + P],
                        bt_sb[:, (g * HC + hc) * H : (g * HC + hc + 1) * H],
                        start=(hc == 0),
                        stop=(hc == HC - 1),
                    )
                tsum = tsump.tile([P, H], F32)
                nc.scalar.copy(tsum[:], ps[:])

                for k in range(K):
                    pd = psd.tile([P, H], F32)
                    for hc in range(HC):
                        xb = (k * HC + hc) * BS + b0
                        nc.tensor.matmul(
                            pd[:],
                            xt_sb[:, xb : xb + P],
                            dt_sb[:, (g * HC + hc) * H : (g * HC + hc + 1) * H],
                            start=(hc == 0),
                            stop=(hc == HC - 1),
                        )
                    ot = opool.tile([P, H], F32)
                    nc.vector.tensor_add(ot[:], pd[:], tsum[:])
                    nc.sync.dma_start(
                        y[b0 : b0 + P, (g * K + k) * H : (g * K + k + 1) * H],
                        ot[:],
                    )

    nc.compile()
    return nc


def _get_nc():
    if "nc" not in _CACHE:
        _CACHE["nc"] = _build()
    return _CACHE["nc"]


def _run(x, A, B, **run_kwargs):
    x = np.ascontiguousarray(np.asarray(x, dtype=np.float32))
    A = np.ascontiguousarray(np.asarray(A, dtype=np.float32))
    B = np.ascontiguousarray(np.asarray(B, dtype=np.float32))

    xt_full = np.ascontiguousarray(x.T)  # [K*H, B_TOTAL]
    s_full = x.reshape(B_TOTAL, K, H).sum(axis=1, dtype=np.float32)
    st_full = np.ascontiguousarray(s_full.T)  # [H, B_TOTAL]
    dtw = np.ascontiguousarray(np.transpose(A - B, (0, 2, 1)))  # [G, h, p]
    btw = np.ascontiguousarray(np.transpose(B, (0, 2, 1)))

    in_maps = []
    for c in range(NCORES):
        cols = slice(c * BS, (c + 1) * BS)
        in_maps.append(
            {
                "xt": np.ascontiguousarray(xt_full[:, cols]),
                "st": np.ascontiguousarray(st_full[:, cols]),
                "dtw": dtw,
                "btw": btw,
            }
        )

    nc = _get_nc()
    res = run_bass_kernel_spmd(nc, in_maps, list(range(NCORES)), **run_kwargs)
    out = np.concatenate([res.results[c]["y"] for c in range(NCORES)], axis=0)
    return out, res


def kernel(x, A, B):
    out, _ = _run(x, A, B)
    return out


# revision 2
# speedup vs baseline: 1.2512x; 1.2512x over previous
"""BlockEqLinear kernel for Trainium2 (8 NeuronCores, SPMD data-parallel over batch).

Math (reference):
    x: [4096, 4096] viewed as [B=4096, K=8, H=512]
    A, B: [G=4, H, H]
    out[b, g, k, :] = x_k[b] @ (A_g - B_g)^T + S[b] @ B_g^T,  S = sum_k x_k
    returned as [B, G*K*H] = [4096, 16384]

Strategy:
  - Shard batch across 8 cores (512 rows each); weights replicated.
  - Host-side layout prep: partition-major transposes so the contraction
    dim (h) is the partition dim on chip; precompute D = A - B.
  - Phase A (per core): tsum[bt,g] = S^T.T @ B_g^T  (16 groups of 4
    accumulating fp32r matmuls), evicted to SBUF.
  - Phase B: k-outer so x^T streams in just-in-time one k-slice at a
    time:  for k: for bt: for g: 4 matmuls -> psum; DVE adds tsum and
    packs 4 g-slices into one staging tile; one DMA per (k, bt).
  - fp32r (FP22) matmuls: full PE rate at moving-dim 512.
  - Output written as bf16 on device (halves write traffic; PE-bound
    after that), upcast to fp32 on host.
"""

import numpy as np

import concourse.bass as bass
import concourse.mybir as mybir
import concourse.tile as tile
from concourse import bacc
from concourse.bass_utils import run_bass_kernel_spmd
from contextlib import ExitStack

G, K, H = 4, 8, 512
B_TOTAL = 4096
NCORES = 8
BS = B_TOTAL // NCORES  # 512 batch rows per core
P = 128                 # partition dim
HC = H // P             # 4 contraction chunks per 512-dim h
NBT = BS // P           # 4 b-tiles per core

F32 = mybir.dt.float32
F32R = mybir.dt.float32r
BF16 = mybir.dt.bfloat16

OUT_BF16 = True         # write y as bf16 on device, upcast on host

_CACHE = {}


def _build():
    out_dt = BF16 if OUT_BF16 else F32

    nc = bacc.Bacc(
        "TRN2", target_bir_lowering=False, debug=False, num_devices=NCORES
    )

    # Host-packed, partition-major inputs:
    #   xt[q, k, hc, b]   = x[b, k*512 + hc*128 + q]
    #   st[q, hc, b]      = S[b, hc*128 + q]
    #   dtw[q, g, hc, p]  = (A - B)[g, p, hc*128 + q]
    #   btw[q, g, hc, p]  = B[g, p, hc*128 + q]
    xt = nc.dram_tensor("xt", [P, K, HC, BS], F32R, kind="ExternalInput")
    st = nc.dram_tensor("st", [P, HC, BS], F32R, kind="ExternalInput")
    dtw = nc.dram_tensor("dtw", [P, G, HC, H], F32R, kind="ExternalInput")
    btw = nc.dram_tensor("btw", [P, G, HC, H], F32R, kind="ExternalInput")
    # y_dev[bt, k, p, g*512 + pp] = out[bt*128 + p, g, k, pp]
    y = nc.dram_tensor("y", [NBT, K, P, G * H], out_dt, kind="ExternalOutput")

    with tile.TileContext(nc) as tc, ExitStack() as ctx:
        wpool = ctx.enter_context(tc.tile_pool(name="w", bufs=1))
        xpool = ctx.enter_context(tc.tile_pool(name="x", bufs=1))
        tsump = ctx.enter_context(tc.tile_pool(name="tsum", bufs=1))
        opool = ctx.enter_context(tc.tile_pool(name="o", bufs=8))
        psd = ctx.enter_context(tc.tile_pool(name="psd", bufs=6, space="PSUM"))
        pss = ctx.enter_context(tc.tile_pool(name="pss", bufs=2, space="PSUM"))

        # SBUF-resident tensors (DMA in consumption order).
        st_sb = xpool.tile([P, HC * BS], F32R)
        nc.sync.dma_start(st_sb[:], st[:, :, :])

        bt_sb = wpool.tile([P, G * HC * H], F32R)
        for g in range(G):
            nc.sync.dma_start(
                bt_sb[:, g * HC * H : (g + 1) * HC * H], btw[:, g, :, :]
            )
        dt_sb = wpool.tile([P, G * HC * H], F32R)
        for g in range(G):
            nc.sync.dma_start(
                dt_sb[:, g * HC * H : (g + 1) * HC * H], dtw[:, g, :, :]
            )
        xt_sb = xpool.tile([P, K * HC * BS], F32R)
        for k in range(K):
            nc.sync.dma_start(
                xt_sb[:, k * HC * BS : (k + 1) * HC * BS], xt[:, k, :, :]
            )

        # Phase A: tsum[bt, g] = S-tile @ B_g^T for all 16 (bt, g) pairs.
        tsum_sb = tsump.tile([P, NBT * G * H], BF16)
        for bt in range(NBT):
            b0 = bt * P
            for g in range(G):
                ps = pss.tile([P, H], F32)
                for hc in range(HC):
                    nc.tensor.matmul(
                        ps[:],
                        st_sb[:, hc * BS + b0 : hc * BS + b0 + P],
                        bt_sb[:, (g * HC + hc) * H : (g * HC + hc + 1) * H],
                        start=(hc == 0),
                        stop=(hc == HC - 1),
                    )
                c = (bt * G + g) * H
                nc.scalar.copy(tsum_sb[:, c : c + H], ps[:])

        # Phase B: k-outer diag matmuls; pack 4 g-slices per (k, bt).
        for k in range(K):
            for bt in range(NBT):
                b0 = bt * P
                ot = opool.tile([P, G * H], out_dt)
                for g in range(G):
                    pd = psd.tile([P, H], F32)
                    for hc in range(HC):
                        xb = (k * HC + hc) * BS + b0
                        nc.tensor.matmul(
                            pd[:],
                            xt_sb[:, xb : xb + P],
                            dt_sb[:, (g * HC + hc) * H : (g * HC + hc + 1) * H],
                            start=(hc == 0),
                            stop=(hc == HC - 1),
                        )
                    c = (bt * G + g) * H
                    nc.vector.tensor_add(
                        ot[:, g * H : (g + 1) * H], pd[:], tsum_sb[:, c : c + H]
                    )
                nc.scalar.dma_start(y[bt, k, :, :], ot[:])

    nc.compile()
    return nc


def _get_nc():
    if "nc" not in _CACHE:
        _CACHE["nc"] = _build()
    return _CACHE["nc"]


def _prep_inputs(x, A, B):
    x = np.ascontiguousarray(np.asarray(x, dtype=np.float32))
    A = np.asarray(A, dtype=np.float32)
    B = np.asarray(B, dtype=np.float32)

    # [q, k, hc, b_global]
    xt_full = np.ascontiguousarray(
        x.T.reshape(K, HC, P, B_TOTAL).transpose(2, 0, 1, 3)
    )
    s_full = x.reshape(B_TOTAL, K, H).sum(axis=1, dtype=np.float32)
    st_full = np.ascontiguousarray(
        s_full.T.reshape(HC, P, B_TOTAL).transpose(1, 0, 2)
    )
    # [q, g, hc, p]
    D = A - B
    dtw = np.ascontiguousarray(
        D.reshape(G, H, HC, P).transpose(3, 0, 2, 1)
    )
    btw = np.ascontiguousarray(
        B.reshape(G, H, HC, P).transpose(3, 0, 2, 1)
    )

    in_maps = []
    for c in range(NCORES):
        cols = slice(c * BS, (c + 1) * BS)
        in_maps.append(
            {
                "xt": np.ascontiguousarray(xt_full[:, :, :, cols]),
                "st": np.ascontiguousarray(st_full[:, :, cols]),
                "dtw": dtw,
                "btw": btw,
            }
        )
    return in_maps


def _unpack_output(res):
    outs = []
    for c in range(NCORES):
        yd = np.asarray(res.results[c]["y"]).astype(np.float32)
        # [bt, k, p, g, pp] -> [bt, p, g, k, pp]
        yc = yd.reshape(NBT, K, P, G, H).transpose(0, 2, 3, 1, 4)
        outs.append(np.ascontiguousarray(yc).reshape(BS, G * K * H))
    return np.concatenate(outs, axis=0)


def _run(x, A, B, **run_kwargs):
    in_maps = _prep_inputs(x, A, B)
    nc = _get_nc()
    res = run_bass_kernel_spmd(nc, in_maps, list(range(NCORES)), **run_kwargs)
    return _unpack_output(res), res


def kernel(x, A, B):
    out, _ = _run(x, A, B)
    return out
